# revision 17
# baseline (speedup 1.0000x reference)
import ml_dtypes
import numpy as np

import concourse.tile as tile
from concourse import bacc, mybir
from concourse.bass_utils import run_bass_kernel_spmd

N_CORES = 8
B, C, H, W = 32, 64, 64, 64
KH = KW = 2
NH, NW = H // KH, W // KW      # 32, 32
L = NH * NW                    # 1024 spatial locations
P = C * KH * KW                # 256 (contraction n == output p)
LC = L // N_CORES              # 128 locations per core
PHC = LC // NW                 # 4 patch-rows per core
HC = H // N_CORES              # 8 image rows per core
G = 8                          # locations per weight-DMA group
NG = LC // G
GO = 16                        # locations per output staging tile
BF16 = ml_dtypes.bfloat16

TRACE = False
LAST_EXEC_NS = None
_nc = None


def _build():
    global _nc
    if _nc is None:
        nc = bacc.Bacc("TRN2", target_bir_lowering=False, debug=False,
                       num_devices=N_CORES)
        xp = nc.dram_tensor("xp", [2, 128, LC * B], mybir.dt.bfloat16,
                            kind="ExternalInput").ap()
        w = nc.dram_tensor("w", [128, NG * 2 * G * P], mybir.dt.bfloat16,
                           kind="ExternalInput").ap()
        out = nc.dram_tensor("out", [B, LC * P], mybir.dt.bfloat16,
                             kind="ExternalOutput").ap()
        F = 2 * G * P
        with tile.TileContext(nc) as tc:
            with tc.tile_pool(name="xpool", bufs=2) as xpool, \
                 tc.tile_pool(name="wpool", bufs=12) as wpool, \
                 tc.tile_pool(name="opool", bufs=2) as opool, \
                 tc.tile_pool(name="warmp", bufs=1) as warmpool, \
                 tc.tile_pool(name="wpsum", bufs=1, space="PSUM") as wppool, \
                 tc.tile_pool(name="psum", bufs=7, space="PSUM") as ppool:
                warm = warmpool.tile([128, B + P], mybir.dt.bfloat16)
                nc.vector.memset(warm[:], 0.0)
                wp_ps = wppool.tile([B, P], mybir.dt.float32)
                for _ in range(40):
                    nc.tensor.matmul(wp_ps[:], lhsT=warm[:, :B],
                                     rhs=warm[:, B:], start=True, stop=True)
                x_sb = []
                for c in range(2):
                    t = xpool.tile([128, LC * B], mybir.dt.bfloat16)
                    nc.sync.dma_start(t[:], xp[c])
                    x_sb.append(t)
                st = None
                for l in range(LC):
                    g, j = divmod(l, G)
                    if j == 0:
                        wt = wpool.tile([128, F], mybir.dt.bfloat16)
                        nc.sync.dma_start(wt[:], w[:, g * F:(g + 1) * F])
                    if l % GO == 0:
                        st = opool.tile([B, GO * P], mybir.dt.bfloat16)
                    pt = ppool.tile([B, P], mybir.dt.float32)
                    for c in range(2):
                        nc.tensor.matmul(
                            pt[:],
                            lhsT=x_sb[c][:, l * B:(l + 1) * B],
                            rhs=wt[:, (c * G + j) * P:(c * G + j + 1) * P],
                            start=(c == 0),
                            stop=(c == 1),
                        )
                    dst = st[:, (l % GO) * P:(l % GO + 1) * P]
                    if l % 2 == 0:
                        nc.vector.tensor_copy(dst, pt[:])
                    else:
                        nc.scalar.copy(dst, pt[:])
                    if l % GO == GO - 1:
                        lo = l - (GO - 1)
                        nc.gpsimd.dma_start(
                            out[:, lo * P:(lo + GO) * P], st[:],
                        )
        nc.compile()
        _nc = nc
    return _nc


def kernel(x, weight, bias):
    nc = _build()
    x = np.asarray(x, dtype=np.float32)
    weight = np.asarray(weight, dtype=np.float32)
    in_maps = []
    for i in range(N_CORES):
        xs = x[:, :, HC * i:HC * (i + 1), :]
        xs = xs.reshape(B, C, PHC, KH, NW, KW)
        xpk = xs.transpose(1, 3, 5, 2, 4, 0).astype(BF16)
        xpk = np.ascontiguousarray(xpk).reshape(2, 128, LC * B)
        wi = weight[LC * i:LC * (i + 1)].reshape(NG, G, 2, 128, P)
        wp = np.ascontiguousarray(wi.transpose(3, 0, 2, 1, 4).astype(BF16))
        in_maps.append({"xp": xpk, "w": wp.reshape(128, NG * 2 * G * P)})
    res = run_bass_kernel_spmd(nc, in_maps, core_ids=list(range(N_CORES)),
                               trace=TRACE)
    global LAST_EXEC_NS
    LAST_EXEC_NS = getattr(res, "exec_time_ns", None)
    out_full = np.empty((B, C, H, W), dtype=np.float32)
    for i in range(N_CORES):
        oi = np.asarray(res.results[i]["out"]).astype(np.float32)
        oi = oi.reshape(B, LC, P).transpose(0, 2, 1)     # [b, p, l]
        oi = oi.reshape(B, C, KH, KW, PHC, NW)
        oi = oi.transpose(0, 1, 4, 2, 5, 3)              # b c ph kh pw kw
        out_full[:, :, HC * i:HC * (i + 1), :] = oi.reshape(B, C, HC, W)
    if bias is not None and np.any(bias):
        bmat = np.asarray(bias, dtype=np.float32).reshape(P, L)
        bf = bmat.reshape(C, KH, KW, NH, NW).transpose(0, 3, 1, 4, 2)
        out_full += bf.reshape(C, H, W)[None]
    return out_full


# revision 22
# speedup vs baseline: 1.0139x; 1.0139x over previous
import ml_dtypes
import numpy as np

import concourse.tile as tile
from concourse import bacc, mybir
from concourse.bass_utils import run_bass_kernel_spmd

N_CORES = 8
B, C, H, W = 32, 64, 64, 64
KH = KW = 2
NH, NW = H // KH, W // KW      # 32, 32
L = NH * NW                    # 1024 spatial locations
P = C * KH * KW                # 256 (contraction n == output p)
LC = L // N_CORES              # 128 locations per core
PHC = LC // NW                 # 4 patch-rows per core
HC = H // N_CORES              # 8 image rows per core
G = 16                         # locations per weight-DMA group
NG = LC // G
GO = 16                        # locations per output staging tile
BF16 = ml_dtypes.bfloat16

TRACE = False
LAST_EXEC_NS = None
_nc = None


def _build():
    global _nc
    if _nc is None:
        nc = bacc.Bacc("TRN2", target_bir_lowering=False, debug=False,
                       num_devices=N_CORES)
        xp = nc.dram_tensor("xp", [2, 128, LC * B], mybir.dt.bfloat16,
                            kind="ExternalInput").ap()
        w = nc.dram_tensor("w", [128, NG * 2 * G * P], mybir.dt.bfloat16,
                           kind="ExternalInput").ap()
        out = nc.dram_tensor("out", [B, LC * P], mybir.dt.bfloat16,
                             kind="ExternalOutput").ap()
        F = 2 * G * P
        with tile.TileContext(nc) as tc:
            with tc.tile_pool(name="xpool", bufs=2) as xpool, \
                 tc.tile_pool(name="wpool", bufs=6) as wpool, \
                 tc.tile_pool(name="opool", bufs=4) as opool, \
                 tc.tile_pool(name="warmp", bufs=1) as warmpool, \
                 tc.tile_pool(name="wpsum", bufs=1, space="PSUM") as wppool, \
                 tc.tile_pool(name="psum", bufs=7, space="PSUM") as ppool:
                warm = warmpool.tile([128, B + P], mybir.dt.bfloat16)
                nc.vector.memset(warm[:], 0.0)
                wp_ps = wppool.tile([B, P], mybir.dt.float32)
                for _ in range(40):
                    nc.tensor.matmul(wp_ps[:], lhsT=warm[:, :B],
                                     rhs=warm[:, B:], start=True, stop=True)
                x_sb = []
                for c in range(2):
                    t = xpool.tile([128, LC * B], mybir.dt.bfloat16)
                    nc.sync.dma_start(t[:], xp[c])
                    x_sb.append(t)
                st = None
                for l in range(LC):
                    g, j = divmod(l, G)
                    if j == 0:
                        wt = wpool.tile([128, F], mybir.dt.bfloat16)
                        nc.sync.dma_start(wt[:], w[:, g * F:(g + 1) * F])
                    if l % GO == 0:
                        st = opool.tile([B, GO * P], mybir.dt.bfloat16)
                    if l % 2 == 0:
                        pt = ppool.tile([B, 2 * P], mybir.dt.float32)
                    half = (l % 2) * P
                    for c in range(2):
                        nc.tensor.matmul(
                            pt[:, half:half + P],
                            lhsT=x_sb[c][:, l * B:(l + 1) * B],
                            rhs=wt[:, (c * G + j) * P:(c * G + j + 1) * P],
                            start=(c == 0),
                            stop=(c == 1),
                        )
                    if l % 2 == 1:
                        dst = st[:, (l - 1) % GO * P:((l - 1) % GO + 2) * P]
                        if (l // 2) % 2 == 0:
                            nc.vector.tensor_copy(dst, pt[:])
                        else:
                            nc.scalar.copy(dst, pt[:])
                    if l % GO == GO - 1:
                        lo = l - (GO - 1)
                        nc.gpsimd.dma_start(
                            out[:, lo * P:(lo + GO) * P], st[:],
                        )
        nc.compile()
        _nc = nc
    return _nc


def kernel(x, weight, bias):
    nc = _build()
    x = np.asarray(x, dtype=np.float32)
    weight = np.asarray(weight, dtype=np.float32)
    in_maps = []
    for i in range(N_CORES):
        xs = x[:, :, HC * i:HC * (i + 1), :]
        xs = xs.reshape(B, C, PHC, KH, NW, KW)
        xpk = xs.transpose(1, 3, 5, 2, 4, 0).astype(BF16)
        xpk = np.ascontiguousarray(xpk).reshape(2, 128, LC * B)
        wi = weight[LC * i:LC * (i + 1)].reshape(NG, G, 2, 128, P)
        wp = np.ascontiguousarray(wi.transpose(3, 0, 2, 1, 4).astype(BF16))
        in_maps.append({"xp": xpk, "w": wp.reshape(128, NG * 2 * G * P)})
    res = run_bass_kernel_spmd(nc, in_maps, core_ids=list(range(N_CORES)),
                               trace=TRACE)
    global LAST_EXEC_NS
    LAST_EXEC_NS = getattr(res, "exec_time_ns", None)
    out_full = np.empty((B, C, H, W), dtype=np.float32)
    for i in range(N_CORES):
        oi = np.asarray(res.results[i]["out"]).astype(np.float32)
        oi = oi.reshape(B, LC, P).transpose(0, 2, 1)     # [b, p, l]
        oi = oi.reshape(B, C, KH, KW, PHC, NW)
        oi = oi.transpose(0, 1, 4, 2, 5, 3)              # b c ph kh pw kw
        out_full[:, :, HC * i:HC * (i + 1), :] = oi.reshape(B, C, HC, W)
    if bias is not None and np.any(bias):
        bmat = np.asarray(bias, dtype=np.float32).reshape(P, L)
        bf = bmat.reshape(C, KH, KW, NH, NW).transpose(0, 3, 1, 4, 2)
        out_full += bf.reshape(C, H, W)[None]
    return out_full


# revision 23
# speedup vs baseline: 1.1700x; 1.1539x over previous
import ml_dtypes
import numpy as np

import concourse.tile as tile
from concourse import bacc, mybir
from concourse.bass_utils import run_bass_kernel_spmd

N_CORES = 8
B, C, H, W = 32, 64, 64, 64
KH = KW = 2
NH, NW = H // KH, W // KW      # 32, 32
L = NH * NW                    # 1024 spatial locations
P = C * KH * KW                # 256 (contraction n == output p)
LC = L // N_CORES              # 128 locations per core
PHC = LC // NW                 # 4 patch-rows per core
HC = H // N_CORES              # 8 image rows per core
G = 16                         # locations per weight-DMA group
NG = LC // G
GO = 16                        # locations per output staging tile
BF16 = ml_dtypes.bfloat16

TRACE = False
LAST_EXEC_NS = None
_nc = None


def _build():
    global _nc
    if _nc is None:
        nc = bacc.Bacc("TRN2", target_bir_lowering=False, debug=False,
                       num_devices=N_CORES)
        xp = nc.dram_tensor("xp", [2, 128, LC * B], mybir.dt.bfloat16,
                            kind="ExternalInput").ap()
        w = nc.dram_tensor("w", [128, NG * 2 * G * P], mybir.dt.bfloat16,
                           kind="ExternalInput").ap()
        out = nc.dram_tensor("out", [B, LC * P], mybir.dt.bfloat16,
                             kind="ExternalOutput").ap()
        F = 2 * G * P
        with tile.TileContext(nc) as tc:
            with tc.tile_pool(name="xpool", bufs=2) as xpool, \
                 tc.tile_pool(name="wpool", bufs=6) as wpool, \
                 tc.tile_pool(name="opool", bufs=8) as opool, \
                 tc.tile_pool(name="warmp", bufs=1) as warmpool, \
                 tc.tile_pool(name="wpsum", bufs=1, space="PSUM") as wppool, \
                 tc.tile_pool(name="psum", bufs=7, space="PSUM") as ppool:
                warm = warmpool.tile([128, B + P], mybir.dt.bfloat16)
                nc.vector.memset(warm[:], 0.0)
                wp_ps = wppool.tile([B, P], mybir.dt.float32)
                for _ in range(40):
                    nc.tensor.matmul(wp_ps[:], lhsT=warm[:, :B],
                                     rhs=warm[:, B:], start=True, stop=True)
                x_sb = []
                for c in range(2):
                    t = xpool.tile([128, LC * B], mybir.dt.bfloat16)
                    nc.sync.dma_start(t[:], xp[c])
                    x_sb.append(t)
                st = None
                for l in range(LC):
                    g, j = divmod(l, G)
                    if j == 0:
                        wt = wpool.tile([128, F], mybir.dt.bfloat16)
                        nc.sync.dma_start(wt[:], w[:, g * F:(g + 1) * F])
                    if l % GO == 0:
                        st = opool.tile([B, GO * P], mybir.dt.bfloat16)
                    if l % 2 == 0:
                        pt = ppool.tile([B, 2 * P], mybir.dt.float32)
                    half = (l % 2) * P
                    for c in range(2):
                        nc.tensor.matmul(
                            pt[:, half:half + P],
                            lhsT=x_sb[c][:, l * B:(l + 1) * B],
                            rhs=wt[:, (c * G + j) * P:(c * G + j + 1) * P],
                            start=(c == 0),
                            stop=(c == 1),
                        )
                    if l % 2 == 1:
                        dst = st[:, (l - 1) % GO * P:((l - 1) % GO + 2) * P]
                        if (l // 2) % 2 == 0:
                            nc.vector.tensor_copy(dst, pt[:])
                        else:
                            nc.scalar.copy(dst, pt[:])
                    if l % GO == GO - 1:
                        lo = l - (GO - 1)
                        nc.gpsimd.dma_start(
                            out[:, lo * P:(lo + GO) * P], st[:],
                        )
        nc.compile()
        _nc = nc
    return _nc


def kernel(x, weight, bias):
    nc = _build()
    x = np.asarray(x, dtype=np.float32)
    weight = np.asarray(weight, dtype=np.float32)
    in_maps = []
    for i in range(N_CORES):
        xs = x[:, :, HC * i:HC * (i + 1), :]
        xs = xs.reshape(B, C, PHC, KH, NW, KW)
        xpk = xs.transpose(1, 3, 5, 2, 4, 0).astype(BF16)
        xpk = np.ascontiguousarray(xpk).reshape(2, 128, LC * B)
        wi = weight[LC * i:LC * (i + 1)].reshape(NG, G, 2, 128, P)
        wp = np.ascontiguousarray(wi.transpose(3, 0, 2, 1, 4).astype(BF16))
        in_maps.append({"xp": xpk, "w": wp.reshape(128, NG * 2 * G * P)})
    res = run_bass_kernel_spmd(nc, in_maps, core_ids=list(range(N_CORES)),
                               trace=TRACE)
    global LAST_EXEC_NS
    LAST_EXEC_NS = getattr(res, "exec_time_ns", None)
    out_full = np.empty((B, C, H, W), dtype=np.float32)
    for i in range(N_CORES):
        oi = np.asarray(res.results[i]["out"]).astype(np.float32)
        oi = oi.reshape(B, LC, P).transpose(0, 2, 1)     # [b, p, l]
        oi = oi.reshape(B, C, KH, KW, PHC, NW)
        oi = oi.transpose(0, 1, 4, 2, 5, 3)              # b c ph kh pw kw
        out_full[:, :, HC * i:HC * (i + 1), :] = oi.reshape(B, C, HC, W)
    if bias is not None and np.any(bias):
        bmat = np.asarray(bias, dtype=np.float32).reshape(P, L)
        bf = bmat.reshape(C, KH, KW, NH, NW).transpose(0, 3, 1, 4, 2)
        out_full += bf.reshape(C, H, W)[None]
    return out_full


# revision 27
# speedup vs baseline: 1.2026x; 1.0279x over previous
import ml_dtypes
import numpy as np

import concourse.tile as tile
from concourse import bacc, mybir
from concourse.bass_utils import run_bass_kernel_spmd

N_CORES = 8
B, C, H, W = 32, 64, 64, 64
KH = KW = 2
NH, NW = H // KH, W // KW      # 32, 32
L = NH * NW                    # 1024 spatial locations
P = C * KH * KW                # 256 (contraction n == output p)
LC = L // N_CORES              # 128 locations per core
PHC = LC // NW                 # 4 patch-rows per core
HC = H // N_CORES              # 8 image rows per core
GS = [16] * 6 + [8] * 4        # weight/staging group sizes (sum = LC)
GMAX = 16
BF16 = ml_dtypes.bfloat16

TRACE = False
LAST_EXEC_NS = None
_nc = None


def _build():
    global _nc
    if _nc is None:
        nc = bacc.Bacc("TRN2", target_bir_lowering=False, debug=False,
                       num_devices=N_CORES)
        xp = nc.dram_tensor("xp", [2, 128, LC * B], mybir.dt.bfloat16,
                            kind="ExternalInput").ap()
        w = nc.dram_tensor("w", [128, 2 * LC * P], mybir.dt.bfloat16,
                           kind="ExternalInput").ap()
        out = nc.dram_tensor("out", [B, LC * P], mybir.dt.bfloat16,
                             kind="ExternalOutput").ap()
        FMAX = 2 * GMAX * P
        with tile.TileContext(nc) as tc:
            with tc.tile_pool(name="xpool", bufs=2) as xpool, \
                 tc.tile_pool(name="wpool", bufs=6) as wpool, \
                 tc.tile_pool(name="opool", bufs=8) as opool, \
                 tc.tile_pool(name="warmp", bufs=1) as warmpool, \
                 tc.tile_pool(name="wpsum", bufs=1, space="PSUM") as wppool, \
                 tc.tile_pool(name="psum", bufs=7, space="PSUM") as ppool:
                warm = warmpool.tile([128, B + P], mybir.dt.bfloat16)
                nc.vector.memset(warm[:], 0.0)
                wp_ps = wppool.tile([B, P], mybir.dt.float32)
                for _ in range(40):
                    nc.tensor.matmul(wp_ps[:], lhsT=warm[:, :B],
                                     rhs=warm[:, B:], start=True, stop=True)
                x_sb = []
                for c in range(2):
                    t = xpool.tile([128, LC * B], mybir.dt.bfloat16)
                    nc.sync.dma_start(t[:], xp[c])
                    x_sb.append(t)
                l = 0
                woff = 0
                for Gg in GS:
                    Fg = 2 * Gg * P
                    wt = wpool.tile([128, FMAX], mybir.dt.bfloat16)
                    nc.sync.dma_start(wt[:, :Fg], w[:, woff:woff + Fg])
                    woff += Fg
                    st = opool.tile([B, GMAX * P], mybir.dt.bfloat16)
                    for j in range(Gg):
                        if j % 2 == 0:
                            pt = ppool.tile([B, 2 * P], mybir.dt.float32)
                        half = (j % 2) * P
                        for c in range(2):
                            nc.tensor.matmul(
                                pt[:, half:half + P],
                                lhsT=x_sb[c][:, l * B:(l + 1) * B],
                                rhs=wt[:, (c * Gg + j) * P:
                                        (c * Gg + j + 1) * P],
                                start=(c == 0),
                                stop=(c == 1),
                            )
                        if j % 2 == 1:
                            dst = st[:, (j - 1) * P:(j + 1) * P]
                            if (l // 2) % 2 == 0:
                                nc.vector.tensor_copy(dst, pt[:])
                            else:
                                nc.scalar.copy(dst, pt[:])
                        l += 1
                    nc.gpsimd.dma_start(
                        out[:, (l - Gg) * P:l * P], st[:, :Gg * P],
                    )
        nc.compile()
        _nc = nc
    return _nc


def kernel(x, weight, bias):
    nc = _build()
    x = np.asarray(x, dtype=np.float32)
    weight = np.asarray(weight, dtype=np.float32)
    in_maps = []
    for i in range(N_CORES):
        xs = x[:, :, HC * i:HC * (i + 1), :]
        xs = xs.reshape(B, C, PHC, KH, NW, KW)
        xpk = xs.transpose(1, 3, 5, 2, 4, 0).astype(BF16)
        xpk = np.ascontiguousarray(xpk).reshape(2, 128, LC * B)
        wi = weight[LC * i:LC * (i + 1)].reshape(LC, 2, 128, P)
        blocks = []
        off_l = 0
        for Gg in GS:
            blk = wi[off_l:off_l + Gg].transpose(2, 1, 0, 3)
            blocks.append(blk.reshape(128, 2 * Gg * P))
            off_l += Gg
        wp = np.concatenate(blocks, axis=1).astype(BF16)
        in_maps.append({"xp": xpk, "w": np.ascontiguousarray(wp)})
    res = run_bass_kernel_spmd(nc, in_maps, core_ids=list(range(N_CORES)),
                               trace=TRACE)
    global LAST_EXEC_NS
    LAST_EXEC_NS = getattr(res, "exec_time_ns", None)
    out_full = np.empty((B, C, H, W), dtype=np.float32)
    for i in range(N_CORES):
        oi = np.asarray(res.results[i]["out"]).astype(np.float32)
        oi = oi.reshape(B, LC, P).transpose(0, 2, 1)     # [b, p, l]
        oi = oi.reshape(B, C, KH, KW, PHC, NW)
        oi = oi.transpose(0, 1, 4, 2, 5, 3)              # b c ph kh pw kw
        out_full[:, :, HC * i:HC * (i + 1), :] = oi.reshape(B, C, HC, W)
    if bias is not None and np.any(bias):
        bmat = np.asarray(bias, dtype=np.float32).reshape(P, L)
        bf = bmat.reshape(C, KH, KW, NH, NW).transpose(0, 3, 1, 4, 2)
        out_full += bf.reshape(C, H, W)[None]
    return out_full
